# revision 19
# baseline (speedup 1.0000x reference)
"""Trainium2 Bass kernel for nn_BasicLSTM (single-step LSTM cell variant).

Reference computation (B=4096, D=1024, S=1024):
    pre_f = inputs @ w_f + h0 @ u_f + b_f
    f     = sigmoid(pre_f)
    i     = sigmoid(inputs @ w_i + h0 @ u_i + b_i)
    o     = sigmoid(inputs @ w_o + h0 @ u_o + b_o)
    c_new = f * c0 + f * i          (input_cell reuses the forget gate)
    h_new = o * tanh(c_new)
    returns (h_new, c_new)

Sharding: batch 4-way x state 2-way over 8 NeuronCores. Core c handles
batch rows [ (c//2)*1024 : (c//2+1)*1024 ) and state cols
[ (c%2)*512 : (c%2+1)*512 ). Host-side prep per core:
    xt  = concat([inputs_rows, h0_rows], 1).T           # [2048, 1024]
    w_g = concat([w_g[:, cols], u_g[:, cols]], 0)       # [2048, 512]
so the device kernel is three plain matmuls (K=2048 contraction on the
partition axis) plus fused elementwise, no on-device transposes.

Schedule notes (from trace analysis; the warm matmul stream runs at the
216ns N=512 issue floor, so all gains are at the edges):
  - The phase-f input stream (xt_k, wf_k pairs) is split over BOTH
    hardware DGE queues in strict k-order, byte-balanced (sync: wf_0 +
    xt_0 second half, odd wf / even xt; scalar: xt_0 first half FIRST,
    odd xt / even wf).  Each queue's early ramp is its own ~150 GB/s
    pipeline; the scalar engine exits its preamble ~0.9us before sync,
    so xt_0-h0 + wf_0 land in parallel ~9.8us and the k=0 round never
    queues behind a single engine's ~0.65us-per-DMA issue stream.
    Every scalar-queue load is consumed within phase f — scalar-queue
    DMAs whose consumers run tens of microseconds after the transfer
    can hard-crash the runtime, as can multi-tile 3D-AP DMAs (both
    found empirically; avoid).
  - Later-deadline loads (bias, wi, c0, wo) follow on the sync queue in
    consumption order; wi/wo/c0 are host-repacked to [128, tiles*n] so
    each loads as one or two plain 2D DMAs (fewer issues/semaphores;
    wi's two 1MB transfers finish ~25us, phase i consumes from ~39us).
  - A memset + N_DUMMY=36 dummy matmuls keep the PE continuously busy
    from each core's preamble end (6.6-7.8us, +-1.1us across cores)
    until first data (~9.8-11.1us) so the HAM clock-gate reaches 8/8
    right as the real stream begins.  Too few dummies leaves a PE idle
    gap that resets the HAM activity window: ~2us of half-clock real
    matmuls (measured both ways; the slowest core sets the score).
  - Phase f is k-major across all 8 PSUM banks.  Phase i runs k-major
    over half-batches (4 btiles at a time); both i-halves stagger their
    last k-rounds so early btiles' ACT chains free PSUM banks sooner.
    Phase o is btile-major; its bias-inject matmuls (K=1 ones x bias,
    opening the last btile's two N=256 groups) are placed after bt2 —
    at bt1 their PSUM slots still wait on phase-i ACT completions and
    stall the PE ~0.3us.
  - c_new (= sig_f * (c0 + sig_i)), its store, and tanh(c_new) are
    computed in phase i; the phase-o tail is only bias+sigmoid+mul+
    store, with the last batch tile as two N=256 PSUM groups and the
    final 64KB store split across both queues (overlapped drains; a
    split of the sigmoid/mul chain itself LOSES ~0.3us/hop to
    inter-engine semaphore latency — keep the tail ops big).
  - c0/biases/outputs travel as fp16 (tolerance 2e-2; fp16 adds ~5e-4).
  - fp8 (e4m3 DoubleRow, ~1.44x PE throughput) was evaluated and is
    numerically DEAD here: quantizing even 2 of 16 k-tiles of one gate
    to e4m3 gives rel err 1.3-2.0e-2 vs the 2e-2 gate (tanh' ~ 1 maps
    pre-activation noise straight onto h); full-fp8 gates measure
    3.7-5.7e-2.  Measured on the real data via work/precision_sim*.py.
"""

import sys

sys.path.insert(0, "/opt/trn_rl_repo")
import ml_dtypes

import numpy as np

B, D, S = 4096, 1024, 1024
N_CORES = 8
BB, SB = 4, 2          # batch blocks x state blocks
B_CORE = B // BB       # 1024 rows per core
S_CORE = S // SB       # 512 state cols per core
K = D + S              # 2048 contraction
KT = K // 128          # 16 k-tiles
BT = B_CORE // 128     # 8 batch tiles per core
N_DUMMY = 40           # PE-warmup matmuls covering the DMA ramp

_CACHE: dict = {}


def _build_nc():
    import concourse.mybir as mybir
    import concourse.tile as tile
    from concourse import bacc

    f32 = mybir.dt.float32
    f16 = mybir.dt.float16

    nc = bacc.Bacc("TRN2", target_bir_lowering=False, debug=False,
                   num_devices=N_CORES)

    xt = nc.dram_tensor("xt", [K, B_CORE], f16, kind="ExternalInput")
    w = {"f": nc.dram_tensor("wf", [K, S_CORE], f16, kind="ExternalInput")}
    # wi/wo host-repacked to [p, kt*n] so each loads as two plain 2D DMAs
    # (fewer queue-engine issue ops; wi is consumed starting ~39us and the
    # two 1MB transfers complete ~25us, so pacing is never at risk)
    w["i"] = nc.dram_tensor("wi", [128, KT * S_CORE], f16,
                            kind="ExternalInput")
    w["o"] = nc.dram_tensor("wo", [128, KT * S_CORE], f16,
                            kind="ExternalInput")
    # biases packed as one [128, 3*S_CORE] broadcast tensor (f, i, o)
    ball = nc.dram_tensor("ball", [128, 3 * S_CORE], f16,
                          kind="ExternalInput")
    # c0 host-repacked to [p, bt*n] -> one 2D DMA
    c0 = nc.dram_tensor("c0", [128, BT * S_CORE], f16, kind="ExternalInput")
    ho = nc.dram_tensor("ho", [B_CORE, S_CORE], f16, kind="ExternalOutput")
    co = nc.dram_tensor("co", [B_CORE, S_CORE], f16, kind="ExternalOutput")

    xt_r = xt.ap().rearrange("(kt p) n -> kt p n", p=128)
    w_r = {"f": w["f"].ap().rearrange("(kt p) n -> kt p n", p=128)}
    ho_r = ho.ap().rearrange("(bt p) n -> bt p n", p=128)
    co_r = co.ap().rearrange("(bt p) n -> bt p n", p=128)

    SIG = mybir.ActivationFunctionType.Sigmoid
    TANH = mybir.ActivationFunctionType.Tanh

    with tile.TileContext(nc) as tc:
        with (
            tc.tile_pool(name="dummyp", bufs=1) as dummyp,
            tc.tile_pool(name="xtp", bufs=KT) as xtp,
            tc.tile_pool(name="wp", bufs=KT) as wp,
            tc.tile_pool(name="wop", bufs=4) as wop,
            tc.tile_pool(name="biasp", bufs=1) as biasp,
            tc.tile_pool(name="c0p", bufs=1) as c0p,
            tc.tile_pool(name="sigfp", bufs=BT) as sigfp,
            tc.tile_pool(name="thp", bufs=BT) as thp,
            tc.tile_pool(name="t1p", bufs=4) as t1p,
            tc.tile_pool(name="cnp", bufs=4) as cnp,
            tc.tile_pool(name="workp", bufs=4) as workp,
            tc.tile_pool(name="psp", bufs=8, space="PSUM") as psp,
        ):
            # ---- PE warmup: memset scratch, then tiny dummy matmuls ----
            dummy = dummyp.tile([128, 128], f16, name="dummy", tag="dummy")
            nc.vector.memset(dummy[:], 1.0)
            ps_d = psp.tile([128, 128], f32, name="ps_dummy", tag="ps")
            for j in range(N_DUMMY):
                nc.tensor.matmul(ps_d[:], dummy[:], dummy[:],
                                 start=True, stop=True)

            w_tiles = {g: [None] * KT for g in "fio"}
            xt_tiles = []

            # ---- phase-f input stream split over BOTH HW queues in strict
            # k-order, byte-balanced (each queue's early ramp is its own
            # ~150 GB/s pipeline, so two queues nearly double supply and
            # the pair-arrival pace beats the 1.73us/k warm consumption).
            # sync: wf_0, xt_0, then odd wf / even xt;  scalar: odd xt /
            # even wf.  Every scalar-queue tile is consumed within phase f
            # and its 15 issues end long before scalar's ACT work.
            # k=0 pair split across BOTH queues so its three transfers ramp
            # in parallel: wf_0 + xt_0-h1 on sync, xt_0-h0 scalar-FIRST
            # (the scalar engine exits its preamble ~0.9us before sync, so
            # h0 lands ~9.8us and bt0-3 can start ~1.3us sooner than when
            # xt_0 queued behind wf_0 on sync).
            wt = wp.tile([128, S_CORE], f16, name="wf_0", tag="w")
            nc.sync.dma_start(out=wt[:], in_=w_r["f"][0])
            w_tiles["f"][0] = wt
            xtt0 = xtp.tile([128, B_CORE], f16, name="xt_0", tag="xt")
            HB = B_CORE // 2
            nc.scalar.dma_start(out=xtt0[:, 0:HB], in_=xt_r[0][:, 0:HB])
            nc.sync.dma_start(out=xtt0[:, HB:], in_=xt_r[0][:, HB:])
            xt_tiles.append(xtt0)
            for k in range(1, KT):
                wt = wp.tile([128, S_CORE], f16, name=f"wf_{k}", tag="w")
                weng = nc.sync if k % 2 == 1 else nc.scalar
                weng.dma_start(out=wt[:], in_=w_r["f"][k])
                w_tiles["f"][k] = wt
                xtt = xtp.tile([128, B_CORE], f16, name=f"xt_{k}", tag="xt")
                xeng = nc.sync if k % 2 == 0 else nc.scalar
                xeng.dma_start(out=xtt[:], in_=xt_r[k])
                xt_tiles.append(xtt)
                if k == 7:
                    bias_all = biasp.tile([128, 3 * S_CORE], f16,
                                          name="bias_all", tag="bias")
                    nc.sync.dma_start(out=bias_all[:], in_=ball.ap())
            bias_t = {g: bias_all[:, gi * S_CORE:(gi + 1) * S_CORE]
                      for gi, g in enumerate("fio")}
            # wi/wo/c0 are host-repacked to [128, tiles*n] so each loads
            # as one or two plain 2D DMAs (fewer issues and semaphores).
            wi_big = []
            for j in range(2):
                wib = wop.tile([128, 8 * S_CORE], f16, name=f"wi_{j}",
                               tag="w")
                nc.sync.dma_start(
                    out=wib[:],
                    in_=w["i"].ap()[:, j * 8 * S_CORE:(j + 1) * 8 * S_CORE])
                wi_big.append(wib)
            for k in range(KT):
                w_tiles["i"][k] = wi_big[k // 8][
                    :, (k % 8) * S_CORE:(k % 8 + 1) * S_CORE]
            c0_all = c0p.tile([128, BT * S_CORE], f16, name="c0_all",
                              tag="c0")
            nc.sync.dma_start(out=c0_all[:], in_=c0.ap())
            c0_t = [c0_all[:, bt * S_CORE:(bt + 1) * S_CORE]
                    for bt in range(BT)]
            wo_big = []
            for j in range(2):
                wob = wop.tile([128, 8 * S_CORE], f16, name=f"wo_{j}",
                               tag="w")
                nc.sync.dma_start(
                    out=wob[:],
                    in_=w["o"].ap()[:, j * 8 * S_CORE:(j + 1) * 8 * S_CORE])
                wo_big.append(wob)
            for k in range(KT):
                w_tiles["o"][k] = wo_big[k // 8][
                    :, (k % 8) * S_CORE:(k % 8 + 1) * S_CORE]

            # ---- phase f: k-major over all 8 PSUM banks ----
            # The last two k rounds are interleaved for bt0..3 so those
            # four banks stop accumulating 1.7-3us before phase-f end:
            # their bias+sigmoid chains (serialized on ACT at ~0.5us each)
            # then free the banks before phase i-a's first k-group needs
            # them, removing the recurring stall at the f->i boundary.
            ps_f = [psp.tile([128, S_CORE], f32, name=f"psf_{bt}", tag="ps")
                    for bt in range(BT)]

            def f_mm(k, bt, stop):
                nc.tensor.matmul(
                    ps_f[bt][:],
                    xt_tiles[k][:, bt * 128:(bt + 1) * 128],
                    w_tiles["f"][k][:],
                    start=(k == 0), stop=stop)

            for k in range(KT - 2):
                for bt in range(BT):
                    f_mm(k, bt, stop=False)
            for bt in range(BT):
                f_mm(KT - 2, bt, stop=False)
                if bt < BT // 2:
                    f_mm(KT - 1, bt, stop=True)
            for bt in range(BT // 2, BT):
                f_mm(KT - 1, bt, stop=True)
            sig_f = []
            for bt in range(BT):
                nc.vector.tensor_add(ps_f[bt][:], ps_f[bt][:], bias_t["f"])
                sf = sigfp.tile([128, S_CORE], f32, name=f"sigf_{bt}",
                                tag="sigf")
                nc.scalar.activation(sf[:], ps_f[bt][:], SIG)
                sig_f.append(sf)

            # ---- phase i: k-major half-batches; finish the c path ----
            th_tiles = [None] * BT

            def i_half(bts, stagger=False):
                bts = list(bts)
                ps = {bt: psp.tile([128, S_CORE], f32, name=f"ps_i_{bt}",
                                   tag="ps") for bt in bts}

                def mm(k, bt, stop):
                    nc.tensor.matmul(
                        ps[bt][:],
                        xt_tiles[k][:, bt * 128:(bt + 1) * 128],
                        w_tiles["i"][k][:],
                        start=(k == 0), stop=stop)

                # staggered final rounds (like phase f's): the first two
                # btiles stop accumulating one k-round early so their
                # bias+sigmoid chains free PSUM banks before phase o's
                # first group needs one (removes the recurring ~0.3us
                # stall at the i->o boundary).
                for k in range(KT - 2):
                    for bt in bts:
                        mm(k, bt, stop=False)
                if stagger:
                    for bt in bts[:2]:
                        mm(KT - 2, bt, stop=False)
                        mm(KT - 1, bt, stop=True)
                    for bt in bts[2:]:
                        mm(KT - 2, bt, stop=False)
                    for bt in bts[2:]:
                        mm(KT - 1, bt, stop=True)
                else:
                    for bt in bts:
                        mm(KT - 2, bt, stop=False)
                    for bt in bts:
                        mm(KT - 1, bt, stop=True)
                for bt in bts:
                    nc.vector.tensor_add(ps[bt][:], ps[bt][:], bias_t["i"])
                    t1 = t1p.tile([128, S_CORE], f32, name=f"t1_{bt}",
                                  tag="t1")
                    nc.scalar.activation(t1[:], ps[bt][:], SIG)
                    nc.vector.tensor_add(t1[:], t1[:], c0_t[bt])
                    cn = cnp.tile([128, S_CORE], f16, name=f"cn_{bt}",
                                  tag="cn")
                    nc.vector.tensor_mul(cn[:], sig_f[bt][:], t1[:])
                    nc.sync.dma_start(out=co_r[bt], in_=cn[:])
                    th = thp.tile([128, S_CORE], f32, name=f"th_{bt}",
                                  tag="th")
                    nc.scalar.activation(th[:], cn[:], TANH)
                    th_tiles[bt] = th

            i_half(range(0, BT // 2))
            i_half(range(BT // 2, BT), stagger=True)

            # ---- phase o: btile-major; tail = bias+sigmoid+mul+store ----
            HS = S_CORE // 2
            ps_o7 = []
            for bt in range(BT - 1):
                ps = psp.tile([128, S_CORE], f32, name=f"ps_o_{bt}",
                              tag="ps")
                for k in range(KT):
                    nc.tensor.matmul(
                        ps[:],
                        xt_tiles[k][:, bt * 128:(bt + 1) * 128],
                        w_tiles["o"][k][:],
                        start=(k == 0), stop=(k == KT - 1))
                nc.vector.tensor_add(ps[:], ps[:], bias_t["o"])
                so = workp.tile([128, S_CORE], f32, name=f"sigo_{bt}",
                                tag="sigo")
                nc.scalar.activation(so[:], ps[:], SIG)
                hn = workp.tile([128, S_CORE], f16, name=f"hn_{bt}",
                                tag="hn")
                nc.vector.tensor_mul(hn[:], so[:], th_tiles[bt][:])
                nc.sync.dma_start(out=ho_r[bt], in_=hn[:])
                if bt == 2:
                    # Hoisted K=1 bias-injection matmuls (ones row x bias
                    # row) open the last batch tile's two accumulation
                    # groups here, off the tail's critical path.  Placed
                    # after bt2's group: at bt1 their PSUM slots still
                    # wait on phase-i ACT completions ($S[165]) and stall
                    # the PE ~0.3us; by bt2 those ACTs are done.
                    for h in range(2):
                        ps7 = psp.tile([128, HS], f32, name=f"ps_o7_{h}",
                                       tag="ps")
                        nc.tensor.matmul(
                            ps7[:], dummy[0:1, :],
                            bias_all[0:1, 2 * S_CORE + h * HS:
                                     2 * S_CORE + (h + 1) * HS],
                            start=True, stop=False)
                        ps_o7.append(ps7)
            # last batch tile: two N=256 groups for a short pipelined tail;
            # the post-matmul chain is just sigmoid+mul+store.
            bt = BT - 1
            for h in range(2):
                c = slice(h * HS, (h + 1) * HS)
                ps = ps_o7[h]
                for k in range(KT):
                    nc.tensor.matmul(
                        ps[:],
                        xt_tiles[k][:, bt * 128:(bt + 1) * 128],
                        w_tiles["o"][k][:, c],
                        start=False, stop=(k == KT - 1))
                so = workp.tile([128, HS], f32, name=f"sigo7_{h}",
                                tag="sigo")
                nc.scalar.activation(so[:], ps[:], SIG)
                hn = workp.tile([128, HS], f16, name=f"hn7_{h}", tag="hn")
                nc.vector.tensor_mul(hn[:], so[:], th_tiles[bt][:, c])
                if h == 0:
                    nc.sync.dma_start(out=ho_r[bt][:, c], in_=hn[:])
                else:
                    # final store split across both queues so the two
                    # 32KB drains overlap (saves ~0.4us on the tail).
                    # Split by PARTITIONS, not columns: per-partition
                    # rows stay 512B = the SDMA line-rate minimum
                    # (<512B packets take a read-modify-write penalty).
                    nc.sync.dma_start(
                        out=ho_r[bt][0:64, c], in_=hn[0:64, :])
                    nc.scalar.dma_start(
                        out=ho_r[bt][64:128, c], in_=hn[64:128, :])

    nc.compile()
    return nc


def _get_nc():
    if "nc" not in _CACHE:
        _CACHE["nc"] = _build_nc()
    return _CACHE["nc"]


def _prep_in_maps(inputs, h0, c0, ws, us, bs):
    """ws/us/bs: dicts g -> full array."""
    in_maps = []
    xts = []
    for blk in range(BB):
        rows = slice(blk * B_CORE, (blk + 1) * B_CORE)
        x = np.concatenate([inputs[rows], h0[rows]], axis=1)  # [1024, 2048]
        xts.append(np.ascontiguousarray(x.T).astype(np.float16))  # [2048, 1024]
    wgs = {}
    for g in "fio":
        for sb in range(SB):
            cols = slice(sb * S_CORE, (sb + 1) * S_CORE)
            wg = np.ascontiguousarray(
                np.concatenate([ws[g][:, cols], us[g][:, cols]],
                               axis=0)).astype(np.float16)
            if g in "io":
                # repack [K, n] -> [128, kt*n] so wi/wo load as 2D DMAs
                wg = np.ascontiguousarray(
                    wg.reshape(KT, 128, S_CORE).transpose(1, 0, 2)
                    .reshape(128, KT * S_CORE))
            wgs[(g, sb)] = wg
    for core in range(N_CORES):
        blk, sb = core // SB, core % SB
        rows = slice(blk * B_CORE, (blk + 1) * B_CORE)
        cols = slice(sb * S_CORE, (sb + 1) * S_CORE)
        bvec = np.concatenate([bs[g][cols] for g in "fio"]).astype(np.float16)
        c0p_ = np.ascontiguousarray(
            c0[rows, cols].astype(np.float16)
            .reshape(BT, 128, S_CORE).transpose(1, 0, 2)
            .reshape(128, BT * S_CORE))
        m = {"xt": xts[blk], "c0": c0p_,
             "ball": np.ascontiguousarray(
                 np.broadcast_to(bvec, (128, 3 * S_CORE)))}
        for g in "fio":
            m[f"w{g}"] = wgs[(g, sb)]
        in_maps.append(m)
    return in_maps


def _run(in_maps, trace=False, trace_kwargs=None, tmpdir=None):
    from concourse.bass_utils import run_bass_kernel_spmd

    nc = _get_nc()
    return run_bass_kernel_spmd(
        nc, in_maps, list(range(N_CORES)), trace=trace,
        trace_kwargs=trace_kwargs or {}, tmpdir=tmpdir,
    )


def _assemble(results):
    h = np.empty((B, S), dtype=np.float32)
    c = np.empty((B, S), dtype=np.float32)
    for core in range(N_CORES):
        blk, sb = core // SB, core % SB
        rows = slice(blk * B_CORE, (blk + 1) * B_CORE)
        cols = slice(sb * S_CORE, (sb + 1) * S_CORE)
        h[rows, cols] = results[core]["ho"].astype(np.float32)
        c[rows, cols] = results[core]["co"].astype(np.float32)
    return h, c


def kernel(inputs, h0, c0, w_f, u_f, b_f, w_i, u_i, b_i, w_o, u_o, b_o):
    inputs = np.asarray(inputs, dtype=np.float32)
    h0 = np.asarray(h0, dtype=np.float32)
    c0 = np.asarray(c0, dtype=np.float32)
    ws = {"f": np.asarray(w_f, np.float32), "i": np.asarray(w_i, np.float32),
          "o": np.asarray(w_o, np.float32)}
    us = {"f": np.asarray(u_f, np.float32), "i": np.asarray(u_i, np.float32),
          "o": np.asarray(u_o, np.float32)}
    bs = {"f": np.asarray(b_f, np.float32), "i": np.asarray(b_i, np.float32),
          "o": np.asarray(b_o, np.float32)}
    in_maps = _prep_in_maps(inputs, h0, c0, ws, us, bs)
    res = _run(in_maps)
    return _assemble(res.results)



# revision 21
# speedup vs baseline: 1.0033x; 1.0033x over previous
"""Trainium2 Bass kernel for nn_BasicLSTM (single-step LSTM cell variant).

Reference computation (B=4096, D=1024, S=1024):
    pre_f = inputs @ w_f + h0 @ u_f + b_f
    f     = sigmoid(pre_f)
    i     = sigmoid(inputs @ w_i + h0 @ u_i + b_i)
    o     = sigmoid(inputs @ w_o + h0 @ u_o + b_o)
    c_new = f * c0 + f * i          (input_cell reuses the forget gate)
    h_new = o * tanh(c_new)
    returns (h_new, c_new)

Sharding: batch 4-way x state 2-way over 8 NeuronCores. Core c handles
batch rows [ (c//2)*1024 : (c//2+1)*1024 ) and state cols
[ (c%2)*512 : (c%2+1)*512 ). Host-side prep per core:
    xt  = concat([inputs_rows, h0_rows], 1).T           # [2048, 1024]
    w_g = concat([w_g[:, cols], u_g[:, cols]], 0)       # [2048, 512]
so the device kernel is three plain matmuls (K=2048 contraction on the
partition axis) plus fused elementwise, no on-device transposes.

Schedule notes (from trace analysis; the warm matmul stream runs at the
216ns N=512 issue floor, so all gains are at the edges):
  - The phase-f input stream (xt_k, wf_k pairs) is split over BOTH
    hardware DGE queues in strict k-order, byte-balanced (sync: wf_0 +
    xt_0 second half, odd wf / even xt; scalar: xt_0 first half FIRST,
    odd xt / even wf).  Each queue's early ramp is its own ~150 GB/s
    pipeline; the scalar engine exits its preamble ~0.9us before sync,
    so xt_0-h0 + wf_0 land in parallel ~9.8us and the k=0 round never
    queues behind a single engine's ~0.65us-per-DMA issue stream.
    Every scalar-queue load is consumed within phase f — scalar-queue
    DMAs whose consumers run tens of microseconds after the transfer
    can hard-crash the runtime, as can multi-tile 3D-AP DMAs (both
    found empirically; avoid).
  - Later-deadline loads (bias, wi, c0, wo) follow on the sync queue in
    consumption order; wi/wo/c0 are host-repacked to [128, tiles*n] so
    each loads as one or two plain 2D DMAs (fewer issues/semaphores;
    wi's two 1MB transfers finish ~25us, phase i consumes from ~39us).
  - A memset + N_DUMMY=36 dummy matmuls keep the PE continuously busy
    from each core's preamble end (6.6-7.8us, +-1.1us across cores)
    until first data (~9.8-11.1us) so the HAM clock-gate reaches 8/8
    right as the real stream begins.  Too few dummies leaves a PE idle
    gap that resets the HAM activity window: ~2us of half-clock real
    matmuls (measured both ways; the slowest core sets the score).
  - Phase f is k-major across all 8 PSUM banks.  Phase i runs k-major
    over half-batches (4 btiles at a time); both i-halves stagger their
    last k-rounds so early btiles' ACT chains free PSUM banks sooner.
    Phase o is btile-major; its bias-inject matmuls (K=1 ones x bias,
    opening the last btile's two N=256 groups) are placed after bt3 —
    any earlier, their PSUM slots still wait on the serial phase-i-b
    ACT chain (ends ~77.5us) and stall the PE ~0.3us.
  - c_new (= sig_f * (c0 + sig_i)), its store, and tanh(c_new) are
    computed in phase i; the phase-o tail is only bias+sigmoid+mul+
    store, with the last batch tile as two N=256 PSUM groups and the
    final 64KB store split BY PARTITION HALVES across both queues
    (overlapped drains at 512B/row line rate — sub-512B rows pay a
    read-modify-write penalty; and a split of the sigmoid/mul chain
    itself LOSES ~0.3us/hop to inter-engine semaphore latency, so the
    tail compute ops stay big).
  - c0/biases/outputs travel as fp16 (tolerance 2e-2; fp16 adds ~5e-4).
  - fp8 (e4m3 DoubleRow, ~1.44x PE throughput) was evaluated and is
    numerically DEAD here: quantizing even 2 of 16 k-tiles of one gate
    to e4m3 gives rel err 1.3-2.0e-2 vs the 2e-2 gate (tanh' ~ 1 maps
    pre-activation noise straight onto h); full-fp8 gates measure
    3.7-5.7e-2.  Measured on the real data via work/precision_sim*.py.
"""

import sys

sys.path.insert(0, "/opt/trn_rl_repo")
import ml_dtypes

import numpy as np

B, D, S = 4096, 1024, 1024
N_CORES = 8
BB, SB = 4, 2          # batch blocks x state blocks
B_CORE = B // BB       # 1024 rows per core
S_CORE = S // SB       # 512 state cols per core
K = D + S              # 2048 contraction
KT = K // 128          # 16 k-tiles
BT = B_CORE // 128     # 8 batch tiles per core
N_DUMMY = 36           # PE-warmup matmuls covering the DMA ramp

_CACHE: dict = {}


def _build_nc():
    import concourse.mybir as mybir
    import concourse.tile as tile
    from concourse import bacc

    f32 = mybir.dt.float32
    f16 = mybir.dt.float16

    nc = bacc.Bacc("TRN2", target_bir_lowering=False, debug=False,
                   num_devices=N_CORES)

    xt = nc.dram_tensor("xt", [K, B_CORE], f16, kind="ExternalInput")
    w = {"f": nc.dram_tensor("wf", [K, S_CORE], f16, kind="ExternalInput")}
    # wi/wo host-repacked to [p, kt*n] so each loads as two plain 2D DMAs
    # (fewer queue-engine issue ops; wi is consumed starting ~39us and the
    # two 1MB transfers complete ~25us, so pacing is never at risk)
    w["i"] = nc.dram_tensor("wi", [128, KT * S_CORE], f16,
                            kind="ExternalInput")
    w["o"] = nc.dram_tensor("wo", [128, KT * S_CORE], f16,
                            kind="ExternalInput")
    # biases packed as one [128, 3*S_CORE] broadcast tensor (f, i, o)
    ball = nc.dram_tensor("ball", [128, 3 * S_CORE], f16,
                          kind="ExternalInput")
    # c0 host-repacked to [p, bt*n] -> one 2D DMA
    c0 = nc.dram_tensor("c0", [128, BT * S_CORE], f16, kind="ExternalInput")
    ho = nc.dram_tensor("ho", [B_CORE, S_CORE], f16, kind="ExternalOutput")
    co = nc.dram_tensor("co", [B_CORE, S_CORE], f16, kind="ExternalOutput")

    xt_r = xt.ap().rearrange("(kt p) n -> kt p n", p=128)
    w_r = {"f": w["f"].ap().rearrange("(kt p) n -> kt p n", p=128)}
    ho_r = ho.ap().rearrange("(bt p) n -> bt p n", p=128)
    co_r = co.ap().rearrange("(bt p) n -> bt p n", p=128)

    SIG = mybir.ActivationFunctionType.Sigmoid
    TANH = mybir.ActivationFunctionType.Tanh

    with tile.TileContext(nc) as tc:
        with (
            tc.tile_pool(name="dummyp", bufs=1) as dummyp,
            tc.tile_pool(name="xtp", bufs=KT) as xtp,
            tc.tile_pool(name="wp", bufs=KT) as wp,
            tc.tile_pool(name="wop", bufs=4) as wop,
            tc.tile_pool(name="biasp", bufs=1) as biasp,
            tc.tile_pool(name="c0p", bufs=1) as c0p,
            tc.tile_pool(name="sigfp", bufs=BT) as sigfp,
            tc.tile_pool(name="thp", bufs=BT) as thp,
            tc.tile_pool(name="t1p", bufs=4) as t1p,
            tc.tile_pool(name="cnp", bufs=4) as cnp,
            tc.tile_pool(name="workp", bufs=4) as workp,
            tc.tile_pool(name="psp", bufs=8, space="PSUM") as psp,
        ):
            # ---- PE warmup: memset scratch, then tiny dummy matmuls ----
            dummy = dummyp.tile([128, 128], f16, name="dummy", tag="dummy")
            nc.vector.memset(dummy[:], 1.0)
            ps_d = psp.tile([128, 128], f32, name="ps_dummy", tag="ps")
            for j in range(N_DUMMY):
                nc.tensor.matmul(ps_d[:], dummy[:], dummy[:],
                                 start=True, stop=True)

            w_tiles = {g: [None] * KT for g in "fio"}
            xt_tiles = []

            # ---- phase-f input stream split over BOTH HW queues in strict
            # k-order, byte-balanced (each queue's early ramp is its own
            # ~150 GB/s pipeline, so two queues nearly double supply and
            # the pair-arrival pace beats the 1.73us/k warm consumption).
            # sync: wf_0, xt_0, then odd wf / even xt;  scalar: odd xt /
            # even wf.  Every scalar-queue tile is consumed within phase f
            # and its 15 issues end long before scalar's ACT work.
            # k=0 pair split across BOTH queues so its three transfers ramp
            # in parallel: wf_0 + xt_0-h1 on sync, xt_0-h0 scalar-FIRST
            # (the scalar engine exits its preamble ~0.9us before sync, so
            # h0 lands ~9.8us and bt0-3 can start ~1.3us sooner than when
            # xt_0 queued behind wf_0 on sync).
            wt = wp.tile([128, S_CORE], f16, name="wf_0", tag="w")
            nc.sync.dma_start(out=wt[:], in_=w_r["f"][0])
            w_tiles["f"][0] = wt
            xtt0 = xtp.tile([128, B_CORE], f16, name="xt_0", tag="xt")
            HB = B_CORE // 2
            nc.scalar.dma_start(out=xtt0[:, 0:HB], in_=xt_r[0][:, 0:HB])
            nc.sync.dma_start(out=xtt0[:, HB:], in_=xt_r[0][:, HB:])
            xt_tiles.append(xtt0)
            for k in range(1, KT):
                wt = wp.tile([128, S_CORE], f16, name=f"wf_{k}", tag="w")
                weng = nc.sync if k % 2 == 1 else nc.scalar
                weng.dma_start(out=wt[:], in_=w_r["f"][k])
                w_tiles["f"][k] = wt
                xtt = xtp.tile([128, B_CORE], f16, name=f"xt_{k}", tag="xt")
                xeng = nc.sync if k % 2 == 0 else nc.scalar
                xeng.dma_start(out=xtt[:], in_=xt_r[k])
                xt_tiles.append(xtt)
                if k == 7:
                    bias_all = biasp.tile([128, 3 * S_CORE], f16,
                                          name="bias_all", tag="bias")
                    nc.sync.dma_start(out=bias_all[:], in_=ball.ap())
            bias_t = {g: bias_all[:, gi * S_CORE:(gi + 1) * S_CORE]
                      for gi, g in enumerate("fio")}
            # wi/wo/c0 are host-repacked to [128, tiles*n] so each loads
            # as one or two plain 2D DMAs (fewer issues and semaphores).
            wi_big = []
            for j in range(2):
                wib = wop.tile([128, 8 * S_CORE], f16, name=f"wi_{j}",
                               tag="w")
                nc.sync.dma_start(
                    out=wib[:],
                    in_=w["i"].ap()[:, j * 8 * S_CORE:(j + 1) * 8 * S_CORE])
                wi_big.append(wib)
            for k in range(KT):
                w_tiles["i"][k] = wi_big[k // 8][
                    :, (k % 8) * S_CORE:(k % 8 + 1) * S_CORE]
            c0_all = c0p.tile([128, BT * S_CORE], f16, name="c0_all",
                              tag="c0")
            nc.sync.dma_start(out=c0_all[:], in_=c0.ap())
            c0_t = [c0_all[:, bt * S_CORE:(bt + 1) * S_CORE]
                    for bt in range(BT)]
            wo_big = []
            for j in range(2):
                wob = wop.tile([128, 8 * S_CORE], f16, name=f"wo_{j}",
                               tag="w")
                nc.sync.dma_start(
                    out=wob[:],
                    in_=w["o"].ap()[:, j * 8 * S_CORE:(j + 1) * 8 * S_CORE])
                wo_big.append(wob)
            for k in range(KT):
                w_tiles["o"][k] = wo_big[k // 8][
                    :, (k % 8) * S_CORE:(k % 8 + 1) * S_CORE]

            # ---- phase f: k-major over all 8 PSUM banks ----
            # The last two k rounds are interleaved for bt0..3 so those
            # four banks stop accumulating 1.7-3us before phase-f end:
            # their bias+sigmoid chains (serialized on ACT at ~0.5us each)
            # then free the banks before phase i-a's first k-group needs
            # them, removing the recurring stall at the f->i boundary.
            ps_f = [psp.tile([128, S_CORE], f32, name=f"psf_{bt}", tag="ps")
                    for bt in range(BT)]

            def f_mm(k, bt, stop):
                nc.tensor.matmul(
                    ps_f[bt][:],
                    xt_tiles[k][:, bt * 128:(bt + 1) * 128],
                    w_tiles["f"][k][:],
                    start=(k == 0), stop=stop)

            for k in range(KT - 2):
                for bt in range(BT):
                    f_mm(k, bt, stop=False)
            for bt in range(BT):
                f_mm(KT - 2, bt, stop=False)
                if bt < BT // 2:
                    f_mm(KT - 1, bt, stop=True)
            for bt in range(BT // 2, BT):
                f_mm(KT - 1, bt, stop=True)
            sig_f = []
            for bt in range(BT):
                nc.vector.tensor_add(ps_f[bt][:], ps_f[bt][:], bias_t["f"])
                sf = sigfp.tile([128, S_CORE], f32, name=f"sigf_{bt}",
                                tag="sigf")
                nc.scalar.activation(sf[:], ps_f[bt][:], SIG)
                sig_f.append(sf)

            # ---- phase i: k-major half-batches; finish the c path ----
            th_tiles = [None] * BT

            def i_half(bts, stagger=False):
                bts = list(bts)
                ps = {bt: psp.tile([128, S_CORE], f32, name=f"ps_i_{bt}",
                                   tag="ps") for bt in bts}

                def mm(k, bt, stop):
                    nc.tensor.matmul(
                        ps[bt][:],
                        xt_tiles[k][:, bt * 128:(bt + 1) * 128],
                        w_tiles["i"][k][:],
                        start=(k == 0), stop=stop)

                # staggered final rounds (like phase f's): the first two
                # btiles stop accumulating one k-round early so their
                # bias+sigmoid chains free PSUM banks before phase o's
                # first group needs one (removes the recurring ~0.3us
                # stall at the i->o boundary).
                for k in range(KT - 2):
                    for bt in bts:
                        mm(k, bt, stop=False)
                if stagger:
                    for bt in bts[:2]:
                        mm(KT - 2, bt, stop=False)
                        mm(KT - 1, bt, stop=True)
                    for bt in bts[2:]:
                        mm(KT - 2, bt, stop=False)
                    for bt in bts[2:]:
                        mm(KT - 1, bt, stop=True)
                else:
                    for bt in bts:
                        mm(KT - 2, bt, stop=False)
                    for bt in bts:
                        mm(KT - 1, bt, stop=True)
                for bt in bts:
                    nc.vector.tensor_add(ps[bt][:], ps[bt][:], bias_t["i"])
                    t1 = t1p.tile([128, S_CORE], f32, name=f"t1_{bt}",
                                  tag="t1")
                    nc.scalar.activation(t1[:], ps[bt][:], SIG)
                    nc.vector.tensor_add(t1[:], t1[:], c0_t[bt])
                    cn = cnp.tile([128, S_CORE], f16, name=f"cn_{bt}",
                                  tag="cn")
                    nc.vector.tensor_mul(cn[:], sig_f[bt][:], t1[:])
                    nc.sync.dma_start(out=co_r[bt], in_=cn[:])
                    th = thp.tile([128, S_CORE], f32, name=f"th_{bt}",
                                  tag="th")
                    nc.scalar.activation(th[:], cn[:], TANH)
                    th_tiles[bt] = th

            i_half(range(0, BT // 2))
            i_half(range(BT // 2, BT), stagger=True)

            # ---- phase o: btile-major; tail = bias+sigmoid+mul+store ----
            HS = S_CORE // 2
            ps_o7 = []
            for bt in range(BT - 1):
                ps = psp.tile([128, S_CORE], f32, name=f"ps_o_{bt}",
                              tag="ps")
                for k in range(KT):
                    nc.tensor.matmul(
                        ps[:],
                        xt_tiles[k][:, bt * 128:(bt + 1) * 128],
                        w_tiles["o"][k][:],
                        start=(k == 0), stop=(k == KT - 1))
                nc.vector.tensor_add(ps[:], ps[:], bias_t["o"])
                so = workp.tile([128, S_CORE], f32, name=f"sigo_{bt}",
                                tag="sigo")
                nc.scalar.activation(so[:], ps[:], SIG)
                hn = workp.tile([128, S_CORE], f16, name=f"hn_{bt}",
                                tag="hn")
                nc.vector.tensor_mul(hn[:], so[:], th_tiles[bt][:])
                nc.sync.dma_start(out=ho_r[bt], in_=hn[:])
                if bt == 3:
                    # Hoisted K=1 bias-injection matmuls (ones row x bias
                    # row) open the last batch tile's two accumulation
                    # groups here, off the tail's critical path.  Placed
                    # after bt3's group: earlier, their PSUM slots still
                    # wait on the serial phase-i-b ACT chain (ends
                    # ~77.5us) and stall the PE ~0.3us.
                    for h in range(2):
                        ps7 = psp.tile([128, HS], f32, name=f"ps_o7_{h}",
                                       tag="ps")
                        nc.tensor.matmul(
                            ps7[:], dummy[0:1, :],
                            bias_all[0:1, 2 * S_CORE + h * HS:
                                     2 * S_CORE + (h + 1) * HS],
                            start=True, stop=False)
                        ps_o7.append(ps7)
            # last batch tile: two N=256 groups for a short pipelined tail;
            # the post-matmul chain is just sigmoid+mul+store.
            bt = BT - 1
            for h in range(2):
                c = slice(h * HS, (h + 1) * HS)
                ps = ps_o7[h]
                for k in range(KT):
                    nc.tensor.matmul(
                        ps[:],
                        xt_tiles[k][:, bt * 128:(bt + 1) * 128],
                        w_tiles["o"][k][:, c],
                        start=False, stop=(k == KT - 1))
                so = workp.tile([128, HS], f32, name=f"sigo7_{h}",
                                tag="sigo")
                nc.scalar.activation(so[:], ps[:], SIG)
                hn = workp.tile([128, HS], f16, name=f"hn7_{h}", tag="hn")
                nc.vector.tensor_mul(hn[:], so[:], th_tiles[bt][:, c])
                if h == 0:
                    nc.sync.dma_start(out=ho_r[bt][:, c], in_=hn[:])
                else:
                    # final store split across both queues so the two
                    # 32KB drains overlap (saves ~0.4us on the tail).
                    # Split by PARTITIONS, not columns: per-partition
                    # rows stay 512B = the SDMA line-rate minimum
                    # (<512B packets take a read-modify-write penalty).
                    nc.sync.dma_start(
                        out=ho_r[bt][0:64, c], in_=hn[0:64, :])
                    nc.scalar.dma_start(
                        out=ho_r[bt][64:128, c], in_=hn[64:128, :])

    nc.compile()
    return nc


def _get_nc():
    if "nc" not in _CACHE:
        _CACHE["nc"] = _build_nc()
    return _CACHE["nc"]


def _prep_in_maps(inputs, h0, c0, ws, us, bs):
    """ws/us/bs: dicts g -> full array."""
    in_maps = []
    xts = []
    for blk in range(BB):
        rows = slice(blk * B_CORE, (blk + 1) * B_CORE)
        x = np.concatenate([inputs[rows], h0[rows]], axis=1)  # [1024, 2048]
        xts.append(np.ascontiguousarray(x.T).astype(np.float16))  # [2048, 1024]
    wgs = {}
    for g in "fio":
        for sb in range(SB):
            cols = slice(sb * S_CORE, (sb + 1) * S_CORE)
            wg = np.ascontiguousarray(
                np.concatenate([ws[g][:, cols], us[g][:, cols]],
                               axis=0)).astype(np.float16)
            if g in "io":
                # repack [K, n] -> [128, kt*n] so wi/wo load as 2D DMAs
                wg = np.ascontiguousarray(
                    wg.reshape(KT, 128, S_CORE).transpose(1, 0, 2)
                    .reshape(128, KT * S_CORE))
            wgs[(g, sb)] = wg
    for core in range(N_CORES):
        blk, sb = core // SB, core % SB
        rows = slice(blk * B_CORE, (blk + 1) * B_CORE)
        cols = slice(sb * S_CORE, (sb + 1) * S_CORE)
        bvec = np.concatenate([bs[g][cols] for g in "fio"]).astype(np.float16)
        c0p_ = np.ascontiguousarray(
            c0[rows, cols].astype(np.float16)
            .reshape(BT, 128, S_CORE).transpose(1, 0, 2)
            .reshape(128, BT * S_CORE))
        m = {"xt": xts[blk], "c0": c0p_,
             "ball": np.ascontiguousarray(
                 np.broadcast_to(bvec, (128, 3 * S_CORE)))}
        for g in "fio":
            m[f"w{g}"] = wgs[(g, sb)]
        in_maps.append(m)
    return in_maps


def _run(in_maps, trace=False, trace_kwargs=None, tmpdir=None):
    from concourse.bass_utils import run_bass_kernel_spmd

    nc = _get_nc()
    return run_bass_kernel_spmd(
        nc, in_maps, list(range(N_CORES)), trace=trace,
        trace_kwargs=trace_kwargs or {}, tmpdir=tmpdir,
    )


def _assemble(results):
    h = np.empty((B, S), dtype=np.float32)
    c = np.empty((B, S), dtype=np.float32)
    for core in range(N_CORES):
        blk, sb = core // SB, core % SB
        rows = slice(blk * B_CORE, (blk + 1) * B_CORE)
        cols = slice(sb * S_CORE, (sb + 1) * S_CORE)
        h[rows, cols] = results[core]["ho"].astype(np.float32)
        c[rows, cols] = results[core]["co"].astype(np.float32)
    return h, c


def kernel(inputs, h0, c0, w_f, u_f, b_f, w_i, u_i, b_i, w_o, u_o, b_o):
    inputs = np.asarray(inputs, dtype=np.float32)
    h0 = np.asarray(h0, dtype=np.float32)
    c0 = np.asarray(c0, dtype=np.float32)
    ws = {"f": np.asarray(w_f, np.float32), "i": np.asarray(w_i, np.float32),
          "o": np.asarray(w_o, np.float32)}
    us = {"f": np.asarray(u_f, np.float32), "i": np.asarray(u_i, np.float32),
          "o": np.asarray(u_o, np.float32)}
    bs = {"f": np.asarray(b_f, np.float32), "i": np.asarray(b_i, np.float32),
          "o": np.asarray(b_o, np.float32)}
    in_maps = _prep_in_maps(inputs, h0, c0, ws, us, bs)
    res = _run(in_maps)
    return _assemble(res.results)

